# revision 8
# baseline (speedup 1.0000x reference)
"""Inverse Hough transform (nn_C_iht) on 8 Trainium2 NeuronCores.

out[n,c,y,x] = sum_a hough[n,c,a, r(a,y,x)]  with a static index table r.

Strategy (per core; batch n is sharded across the 8 cores, c=128 channels sit
on the SBUF partition axis):
  - The gather-sum is evaluated as a sequence of one-hot matmuls on the
    TensorEngine.  For a pixel block P (8 wide x 16 tall = 128 pixels) and a
    chunk C = (16 consecutive angles) x (8 consecutive rhos), K = 128:
        psum[c, px] += Hp_chunk[k, c].T  @  E_chunk[k, px]
    where Hp_chunk is an affine slice of a host-side rectangle re-layout of
    the input and E_chunk in {0,1} is the (static) one-hot selector
    E[(ai,rj), px] = [ r(a, px) == rho ].
  - E is NOT streamed from HBM (that made the baseline DMA-bound).  Instead a
    per-(block,group) relative index table D[p,px] = r(a(p),px) - 8*rlo is
    streamed as int16 and E is generated on-chip on the Vector engine:
        E = ((D - rj) == 8*k)   via one scalar_tensor_tensor per size class
    with a static iota tile for the 8*k ramp.  All operands are 2-byte,
    SBUF-resident, stride-1 in the last dim -> DVE runs in 4x mode
    (4 elem/cycle/lane), so E-gen (~1.0M cols) costs ~260-300us, hidden
    under the ~420us of TensorE matmul work.  DMA drops 152MB -> ~101MB.
  - PSUM drains run on the Scalar (Act) engine to keep the DVE free.
"""

import sys

sys.path.insert(0, "/opt/trn_rl_repo")

import numpy as np
import ml_dtypes

N, C, HIMG, WIMG = 8, 128, 160, 160
NUMANGLE, NUMRHO = 180, 180

# chunk geometry
G = 16         # angles per chunk
B = 8          # rhos per chunk  (G * B = 128 = contraction dim)
APAD = 192     # padded angle count  (12 groups of 16)
RPAD = 192     # padded rho count    (24 rho-blocks of 8)
NG = APAD // G          # 12 angle groups
NR = RPAD // B          # 24 rho blocks
BW, BH = 8, 16          # pixel block: 8 wide (x), 16 tall (y) -> 128 px
PX = BW * BH
NBX, NBY = WIMG // BW, HIMG // BH   # 20 x 10 = 200 blocks
XGRP = 2                # blocks per slab (output staging + E-gen batch)
TPS = XGRP * NG         # D tiles per slab (24)
NSLAB = NBY * (NBX // XGRP)
MCAP = 8                # max tiles per E-gen class op (3D AP: [p, nch, m*PX])

BF16 = ml_dtypes.bfloat16
PAD_D = 20000           # D value for padded angle rows (never matches iota)

_rint = lambda v: int(v)


def _rho_table() -> np.ndarray:
    """Exact replica of the reference's index table r[a, y, x]."""
    irho = (int(np.sqrt(HIMG * HIMG + WIMG * WIMG)) + 1) / float(NUMRHO)
    itheta = np.pi / NUMANGLE
    theta = np.arange(NUMANGLE) * itheta
    tab_cos = np.cos(theta) / irho
    tab_sin = np.sin(theta) / irho
    xs = np.arange(WIMG) - WIMG // 2
    ys = np.arange(HIMG) - HIMG // 2
    r = np.round(xs[None, None, :] * tab_cos[:, None, None]
                 + ys[None, :, None] * tab_sin[:, None, None]).astype(np.int64)
    return np.clip(r + NUMRHO // 2, 0, NUMRHO - 1)  # [A, H, W]


def _build_schedule():
    """Static slab schedule + packed int16 D stream.

    Returns (slabs, d_stream, nch_max, emax):
      slabs: list (by-major, then bx-slab) of dicts with
        'classes': [(m, nch, tile_idx0)]  stt ops, tiles sorted by nch
        'blocks':  [[(g, rlo, nch, e_off) x NG] x XGRP]  consumption order
      d_stream: [128, NSLAB*TPS*PX] int16, one [128, PX] tile per schedule
        entry in class-sorted order.
    """
    R = _rho_table()
    slabs = []
    d_parts = []
    nch_max = 0
    emax = 0
    for by in range(NBY):
        for bxg in range(NBX // XGRP):
            tiles = []  # (nch, bxi, g, rlo, D)
            for bxi in range(XGRP):
                bx = bxg * XGRP + bxi
                sub = R[:, by * BH:(by + 1) * BH, bx * BW:(bx + 1) * BW]
                sub = sub.reshape(NUMANGLE, PX)  # px = dy*BW + dx
                for g in range(NG):
                    a0, a1 = g * G, min((g + 1) * G, NUMANGLE)
                    asub = sub[a0:a1]
                    rlo = int(asub.min()) // B
                    nch = int(asub.max()) // B - rlo + 1
                    d = np.full((128, PX), PAD_D, np.int16)
                    na = a1 - a0
                    # partition p = ai*B + rj holds r(a0+ai, px) - 8*rlo
                    rep = np.repeat(asub - rlo * B, B, axis=0).astype(np.int16)
                    d[:na * B] = rep
                    tiles.append((nch, bxi, g, rlo, d))
            tiles.sort(key=lambda t: t[0])
            # classes: runs of equal nch, capped at MCAP tiles per op.
            # E layout per class op (3D AP): [p, nch, m*PX]; tile i of the
            # class has its chunk k at column  e_base + (k*m + i) * PX.
            classes = []
            blocks = [[None] * NG for _ in range(XGRP)]
            e_off = 0
            i = 0
            while i < len(tiles):
                j = min(len(tiles), i + MCAP)
                while j > i and tiles[j - 1][0] != tiles[i][0]:
                    j -= 1
                m, nch = j - i, tiles[i][0]
                classes.append((m, nch, i, e_off))
                for ii in range(i, j):
                    _, bxi, g, rlo, d = tiles[ii]
                    blocks[bxi][g] = (g, rlo, nch, e_off, m, ii - i)
                    d_parts.append(d)
                e_off += m * nch
                nch_max = max(nch_max, nch)
                i = j
            emax = max(emax, e_off)
            slabs.append({"classes": classes, "blocks": blocks})
    d_stream = np.ascontiguousarray(
        np.concatenate(d_parts, axis=1))  # [128, NSLAB*TPS*PX]
    assert d_stream.shape == (128, NSLAB * TPS * PX)
    return slabs, d_stream, nch_max, emax


def _pack_h(h_core: np.ndarray) -> np.ndarray:
    """[C, A, RHO] fp32 -> rectangle layout [128, NG*NR*128] bf16.

    Hp[ai*B+rj, ((g*NR)+r)*128 + c] = h[c, g*G+ai, r*B+rj]
    """
    hp = np.zeros((C, APAD, RPAD), np.float32)
    hp[:, :NUMANGLE, :NUMRHO] = h_core
    hp = hp.reshape(C, NG, G, NR, B)
    hp = hp.transpose(2, 4, 1, 3, 0)           # [G, B, NG, NR, C]
    return np.ascontiguousarray(hp.reshape(G * B, NG * NR * C).astype(BF16))


_SCHED_CACHE = None


def _schedule():
    global _SCHED_CACHE
    if _SCHED_CACHE is None:
        _SCHED_CACHE = _build_schedule()
    return _SCHED_CACHE


def _rj_col() -> np.ndarray:
    return (np.arange(128, dtype=np.int16) % B).reshape(128, 1)


def prepare_inputs(hough_feat: np.ndarray) -> list[dict]:
    _, d_stream, _, _ = _schedule()
    rj = _rj_col()
    return [{"hp": _pack_h(hough_feat[i].astype(np.float32)),
             "d": d_stream, "rj": rj} for i in range(N)]


def build_bass(reps: int = 1):
    """Build the Bass program (single-core SPMD; same program on all cores)."""
    import concourse.mybir as mybir
    from concourse import bacc
    from concourse.tile import TileContext

    slabs, d_stream, nch_max, emax = _schedule()

    nc = bacc.Bacc(None, target_bir_lowering=False)
    hp_d = nc.dram_tensor("hp", [128, NG * NR * C], mybir.dt.bfloat16,
                          kind="ExternalInput")
    d_d = nc.dram_tensor("d", [128, NSLAB * TPS * PX], mybir.dt.int16,
                         kind="ExternalInput")
    rj_d = nc.dram_tensor("rj", [128, 1], mybir.dt.int16,
                          kind="ExternalInput")
    out_d = nc.dram_tensor("out", [128, HIMG * WIMG], mybir.dt.float32,
                           kind="ExternalOutput")

    with TileContext(nc) as tc:
        with tc.tile_pool(name="hp_pool", bufs=1) as hp_pool, \
             tc.tile_pool(name="const_pool", bufs=1) as const_pool, \
             tc.tile_pool(name="d_pool", bufs=2) as d_pool, \
             tc.tile_pool(name="e_pool", bufs=2) as e_pool, \
             tc.tile_pool(name="stage_pool", bufs=3) as stage_pool, \
             tc.tile_pool(name="psum_pool", bufs=8, space="PSUM") as psum_pool:
            hp_t = hp_pool.tile([128, NG * NR * C], mybir.dt.bfloat16)
            nc.sync.dma_start(hp_t[:], hp_d[:])
            rj_t = const_pool.tile([128, 1], mybir.dt.int16)
            nc.sync.dma_start(rj_t[:], rj_d[:])
            iw = MCAP * PX
            iota_t = const_pool.tile([128, nch_max * iw], mybir.dt.int16)
            nc.gpsimd.iota(
                iota_t[:].rearrange("p (n w) -> p n w", n=nch_max),
                pattern=[[B, nch_max], [0, iw]],
                base=0, channel_multiplier=0)

            for _ in range(reps):
                si = 0
                for by in range(NBY):
                    for bxg in range(NBX // XGRP):
                        slab = slabs[si]
                        dt_ = d_pool.tile([128, TPS * PX], mybir.dt.int16,
                                          tag="d")
                        nc.sync.dma_start(
                            dt_[:], d_d[:, si * TPS * PX:(si + 1) * TPS * PX])
                        et = e_pool.tile([128, emax * PX], mybir.dt.bfloat16,
                                         tag="e")
                        for (m, nch, t0, e0) in slab["classes"]:
                            w = m * PX
                            out_ap = et[:, e0 * PX:e0 * PX + nch * w] \
                                .rearrange("p (n w) -> p n w", n=nch)
                            in0 = dt_[:, t0 * PX:t0 * PX + w] \
                                .unsqueeze(1).broadcast_to((128, nch, w))
                            in1 = iota_t[:].rearrange(
                                "p (n w) -> p n w", n=nch_max)[:, :nch, :w]
                            nc.vector.scalar_tensor_tensor(
                                out_ap, in0, rj_t[:, 0:1], in1,
                                op0=mybir.AluOpType.subtract,
                                op1=mybir.AluOpType.is_equal)
                        stage = stage_pool.tile([128, XGRP * PX],
                                                mybir.dt.float32, tag="stage")
                        for bxi in range(XGRP):
                            ps = psum_pool.tile([128, PX], mybir.dt.float32,
                                                tag="ps")
                            chunks = []
                            for (g, rlo, nch, e0, m, ti) in slab["blocks"][bxi]:
                                for k in range(nch):
                                    chunks.append(((g * NR + rlo + k) * C,
                                                   (e0 + k * m + ti) * PX))
                            nch_tot = len(chunks)
                            for ci, (col, eoff) in enumerate(chunks):
                                nc.tensor.matmul(
                                    ps[:],
                                    hp_t[:, col:col + C],
                                    et[:, eoff:eoff + PX],
                                    start=(ci == 0),
                                    stop=(ci == nch_tot - 1),
                                )
                            nc.scalar.copy(
                                stage[:].rearrange(
                                    "p (dy bxs dx) -> p dy bxs dx",
                                    dy=BH, bxs=XGRP)[:, :, bxi, :],
                                ps[:].rearrange("p (dy dx) -> p dy dx", dy=BH),
                            )
                        bx0 = bxg * XGRP
                        dst = out_d[:].rearrange("p (y x) -> p y x", y=HIMG)
                        nc.sync.dma_start(
                            dst[:, by * BH:(by + 1) * BH,
                                bx0 * BW:(bx0 + XGRP) * BW],
                            stage[:].rearrange("p (dy x) -> p dy x", dy=BH),
                        )
                        si += 1
    nc.compile()
    return nc


def _run(nc, in_maps, n_cores):
    from concourse.bass_utils import run_bass_kernel_spmd
    return run_bass_kernel_spmd(nc, in_maps, core_ids=list(range(n_cores)))


def kernel(hough_feat: np.ndarray) -> np.ndarray:
    hough_feat = np.asarray(hough_feat)
    assert hough_feat.shape == (N, C, NUMANGLE, NUMRHO)
    nc = build_bass(reps=1)
    in_maps = prepare_inputs(hough_feat)
    res = _run(nc, in_maps, N)
    out = np.stack([r["out"].reshape(C, HIMG, WIMG) for r in res.results])
    return out.astype(hough_feat.dtype, copy=False)


if __name__ == "__main__":
    slabs, d_stream, nch_max, emax = _schedule()
    tot = sum(t[2] for s in slabs for blk in s["blocks"] for t in blk)
    ncls = sum(len(s["classes"]) for s in slabs)
    print(f"chunks total={tot} nch_max={nch_max} emax={emax} "
          f"classes={ncls} D MB={d_stream.nbytes/1e6:.1f}")
